# revision 22
# baseline (speedup 1.0000x reference)
"""MultiHeadAttention kernel for 8x TRN2 NeuronCores.

The reference module's einsum reduces the attention tensor over BOTH the
query and key axes (attn_mass = sum_{q,k} softmax(logits)_k), and softmax
rows sum to 1, so attn_mass == Lq exactly for every (batch, head). The
whole computation therefore collapses to

    out = (Lq * (V_heads @ Wv^T + bv)).reshape(N, L, E) @ Wo^T + bo

which is a single dense GEMM after folding the (block-diagonal) per-head
V-projection into the output projection:

    out = V_flat @ W_eff + b_eff
    W_eff[h*hd+a, n] = Lq * sum_b Wv[b, a] * Wo[n, h*hd+b]      (1024 x 1024)
    b_eff[n]         = Lq * sum_{h,b} Wo[n, h*hd+b] * bv[b] + bo[n]

The device kernel is the GEMM, row-sharded across 8 cores (512 rows per
core), computed in TRANSPOSED orientation: out^T[n, m] = sum_k W[k, n]
X[m, k].  PSUM bank j holds output columns j*128..(j+1)*128 on partitions
x all 512 rows on the free dim, accumulating lhsT = W-block j against
rhs = X^T k-slabs.

v2 (this file): everything rides bf16 (inputs, weights, output — PSUM
still accumulates fp32; 2e-2 tolerance leaves ~5x margin), halving HBM
traffic, and the schedule is rebuilt around the two real bottlenecks the
fp32 trace exposed:

  * HAM clock ramp: the PE runs at ~1.2 GHz until it has been
    continuously busy ~4us, and a mid-stream DMA stall re-cools it
    (the fp32 run paid ~10us at half clock after stalling).  So: a
    bf16 junk-matmul burst starts the ramp right after the preamble
    and is sized so the first real matmul's inputs have landed by the
    time it drains — the PE never idles once started.
  * DMA supply: inputs stream over THREE queues (sync HWDGE, scalar
    HWDGE, gpsimd SWDGE), each tile ordered by its consumption
    deadline.  Banks 0 and 1 are interleaved (k-offset 2) so the
    X-slab consumption rate during the arrival phase is halved.
  * Output is bf16 too (host upcasts): banks evict through the vector
    engine (bias add fused, fp32->bf16) and drain on the sync queue,
    with the last bank split into quarters to shave the tail.
"""

import numpy as np
import ml_dtypes

import concourse.bass as bass
import concourse.bacc as bacc
import concourse.mybir as mybir
from concourse.tile import TileContext
from concourse.bass_utils import run_bass_kernel_spmd

N_CORES = 8
E = 1024            # embed dim == d_model
H, HD = 16, 64      # heads, head dim
ROWS = 4096         # N * L = 2 * 2048
RPC = ROWS // N_CORES   # rows per core = 512
P = 128             # SBUF partitions
KT = E // P         # 8 contraction slabs
JT = E // P         # 8 output-column banks

# Junk-matmul warm-up burst: keeps the PE busy (HAM ramp) from preamble
# exit until the first real operands land (~3.4us at the mid p-state).
N_JUNK_512 = 9
N_JUNK_128 = 2

# MM order = availability-greedy against the MEASURED per-DMA ready
# times (all three queues go quiet in the ~15.5-17.2us chip-wide
# contention window, so bank 7 -- whose W halves land right after it --
# fills that hole, and the late X tails x6/x7 are consumed as late as
# possible).  Bank 6 is last.
MM_ORDER = [
    (0, 0), (0, 1), (0, 2), (1, 0), (1, 1), (1, 2), (0, 3), (1, 3),
    (2, 0), (2, 1), (2, 2), (2, 3), (0, 4), (0, 5), (7, 0), (7, 1),
    (7, 2), (7, 3), (0, 6), (1, 4), (1, 5), (0, 7), (1, 6), (1, 7),
    (7, 4), (7, 5), (7, 6), (7, 7), (3, 0), (3, 1), (3, 2), (3, 3),
    (2, 4), (2, 5), (2, 6), (2, 7), (3, 4), (3, 5), (3, 6), (3, 7),
] + [(j, k) for j in (4, 5, 6) for k in range(KT)]
# Bank completion order implied by MM_ORDER (evictions follow it).
EVICT_ORDER = [0, 1, 7, 2, 3, 4, 5, 6]

_NC_CACHE = {}
LAST_RESULTS = None  # BassKernelResults of the most recent device run


def _build():
    f32 = mybir.dt.float32
    bf16 = mybir.dt.bfloat16
    nc = bacc.Bacc(None, target_bir_lowering=False)
    xs = nc.declare_dram_parameter("xs", [E, RPC], bf16, isOutput=False)
    wc = nc.declare_dram_parameter("wc", [JT * P, E], bf16, isOutput=False)
    bw = nc.declare_dram_parameter("bw", [P, JT], f32, isOutput=False)
    outT = nc.declare_dram_parameter("outT", [E, RPC], bf16, isOutput=True)

    with TileContext(nc) as tc:
        with (
            tc.tile_pool(name="xp", bufs=1) as xp,
            tc.tile_pool(name="wp", bufs=1) as wp,
            tc.tile_pool(name="bp", bufs=1) as bp,
            tc.tile_pool(name="pp", bufs=1, space="PSUM") as pp,
            tc.tile_pool(name="op", bufs=1) as op,
        ):
            # Junk tile for the warm-up burst: memset needs no DMA and runs
            # first on gpsimd, so the PE can start right after the preamble
            # (a vector-side memset was measured to delay the burst ~1us).
            wm = bp.tile([P, RPC], bf16, name="wm", tag="wm")
            nc.gpsimd.memset(wm[:], 1.0)
            bias = bp.tile([P, JT], f32, name="bias", tag="bias")

            xts = [
                xp.tile([P, RPC], bf16, name=f"x{k}", tag=f"x{k}")
                for k in range(KT)
            ]

            def xslab(k):
                return xts[k][:, :]

            # W chunk tiles; wmap[(j, k)] = (tile, col offset).
            wmap = {}

            def wchunk(j, k0, k1, engine):
                t = wp.tile([P, (k1 - k0) * P], bf16, name=f"w{j}_{k0}{k1}",
                            tag=f"w{j}_{k0}{k1}")
                engine.dma_start(
                    out=t[:], in_=wc[j * P:(j + 1) * P, k0 * P:k1 * P]
                )
                for k in range(k0, k1):
                    wmap[(j, k)] = (t, (k - k0) * P)

            # --- DMA schedule ------------------------------------------
            # The v4-measured queue cadences are kept verbatim (first
            # piece ready ~2.5us after issue, then ~1.2-3us per 128KB
            # piece under full 8-core contention); only the late pieces
            # are reordered: w7's halves land in the contention hole so
            # bank 7 can run there, x6 follows.
            wchunk(0, 0, 4, nc.gpsimd)
            wchunk(1, 0, 4, nc.gpsimd)
            wchunk(2, 0, 4, nc.gpsimd)
            wchunk(7, 0, 4, nc.gpsimd)
            nc.gpsimd.dma_start(out=xts[6][:], in_=xs[6 * P:7 * P, :])
            wchunk(7, 4, 8, nc.gpsimd)
            # sync HWDGE: x0 first (gates the first real MM), x1, x4,
            # then the W0/W1 second halves and W3/W5.
            nc.sync.dma_start(out=xts[0][:], in_=xs[0:P, :])
            nc.sync.dma_start(out=xts[1][:], in_=xs[P:2 * P, :])
            nc.sync.dma_start(out=xts[4][:], in_=xs[4 * P:5 * P, :])
            wchunk(0, 4, 8, nc.sync)
            wchunk(1, 4, 8, nc.sync)
            wchunk(3, 0, 4, nc.sync)
            wchunk(3, 4, 8, nc.sync)
            wchunk(5, 0, 4, nc.sync)
            wchunk(5, 4, 8, nc.sync)
            # scalar HWDGE: bias (tiny, unblocks evictions), its X share,
            # the W2 second half, then W4/W6.
            nc.scalar.dma_start(out=bias[:], in_=bw[:, :])
            nc.scalar.dma_start(out=xts[2][:], in_=xs[2 * P:3 * P, :])
            nc.scalar.dma_start(out=xts[3][:], in_=xs[3 * P:4 * P, :])
            nc.scalar.dma_start(out=xts[5][:], in_=xs[5 * P:6 * P, :])
            nc.scalar.dma_start(out=xts[7][:], in_=xs[7 * P:8 * P, :])
            wchunk(2, 4, 8, nc.scalar)
            wchunk(4, 0, 4, nc.scalar)
            wchunk(4, 4, 8, nc.scalar)
            wchunk(6, 0, 4, nc.scalar)
            wchunk(6, 4, 8, nc.scalar)

            ps = [
                pp.tile([P, RPC], f32, name=f"ps{j}", tag=f"ps{j}")
                for j in range(JT)
            ]

            # Warm-up burst: nonzero bf16 junk matmuls, no DMA deps.
            for i in range(N_JUNK_512):
                nc.tensor.matmul(
                    ps[i % JT], wm[:, 0:P], wm[:, :], start=True, stop=True
                )
            for i in range(N_JUNK_128):
                nc.tensor.matmul(
                    ps[(N_JUNK_512 + i) % JT][:, 0:P],
                    wm[:, 0:P], wm[:, 0:P], start=True, stop=True,
                )

            for j, k in MM_ORDER:
                t, off = wmap[(j, k)]
                nc.tensor.matmul(
                    ps[j],
                    t[:, off:off + P],
                    xslab(k),
                    start=(k == 0),
                    stop=(k == KT - 1),
                )

            # Evictions in bank-completion order: fused bias add
            # fp32->bf16 on vector, out DMAs routed to whichever queue is
            # drained when the bank completes (each engine's outs queue
            # naturally behind its remaining input FIFO traffic).  The
            # LAST bank (6) evicts split BY PARTITION (full 1KB DRAM
            # rows, unlike column halves whose 512B strided pieces
            # transfer ~2x slower) on vector + scalar-activation, with
            # out DMAs on sync + scalar in parallel.
            hp = P // 2
            out_eng = {0: nc.gpsimd, 1: nc.gpsimd, 7: nc.scalar,
                       2: nc.gpsimd, 3: nc.gpsimd, 4: nc.sync,
                       5: nc.scalar}
            for j in EVICT_ORDER[:-1]:
                o = op.tile([P, RPC], bf16, name=f"o{j}", tag=f"o{j}")
                nc.vector.tensor_scalar_add(o[:], ps[j], bias[:, j:j + 1])
                out_eng[j].dma_start(
                    out=outT[j * P:(j + 1) * P, :], in_=o[:]
                )
            o6 = op.tile([P, RPC], bf16, name="o6", tag="o6")
            nc.vector.tensor_scalar_add(o6[0:hp, :], ps[6][0:hp, :],
                                        bias[0:hp, 6:7])
            nc.sync.dma_start(out=outT[6 * P:6 * P + hp, :], in_=o6[0:hp, :])
            nc.scalar.add(o6[hp:P, :], ps[6][hp:P, :], bias[hp:P, 6:7])
            nc.scalar.dma_start(out=outT[6 * P + hp:7 * P, :], in_=o6[hp:P, :])
    nc.compile()
    return nc


def _get_nc():
    if "bf16" not in _NC_CACHE:
        _NC_CACHE["bf16"] = _build()
    return _NC_CACHE["bf16"]


def _prep_in_maps(V, Wv, bv, Wo, bo, lq):
    V = np.ascontiguousarray(np.asarray(V, dtype=np.float32))
    Wv64 = np.asarray(Wv, np.float64)
    Wo64 = np.asarray(Wo, np.float64)
    bv64 = np.asarray(bv, np.float64)
    bo64 = np.asarray(bo, np.float64)

    # Fold per-head V-projection + output projection + attention mass (== Lq).
    Wo_r = Wo64.reshape(E, H, HD)                       # [n, h, b]
    W_eff = lq * np.einsum("ba,nhb->han", Wv64, Wo_r, optimize=True)
    W_eff = W_eff.reshape(E, E).astype(np.float32)      # [k, n]
    b_eff = (lq * np.einsum("nhb,b->n", Wo_r, bv64) + bo64).astype(np.float32)

    # wc[j*P + p, k*P + c] = W_eff[k*P + p, j*P + c]  (lhsT blocks, natural)
    wc = np.ascontiguousarray(
        W_eff.reshape(KT, P, JT, P).transpose(2, 1, 0, 3).reshape(JT * P, E)
    ).astype(ml_dtypes.bfloat16)
    bw_blk = np.ascontiguousarray(b_eff.reshape(JT, P).T)   # [p, j] fp32

    X = V.reshape(ROWS, E)
    in_maps = []
    for i in range(N_CORES):
        xs_i = np.ascontiguousarray(
            X[i * RPC:(i + 1) * RPC, :].T.astype(ml_dtypes.bfloat16)
        )
        in_maps.append({"xs": xs_i, "wc": wc, "bw": bw_blk})
    return in_maps


def kernel(Q, K, V, Wq, bq, Wk, bk, Wv, bv, Wo, bo, **_unused):
    global LAST_RESULTS
    n, L, e = np.asarray(V).shape
    lq = float(np.asarray(Q).shape[1])
    in_maps = _prep_in_maps(V, Wv, bv, Wo, bo, lq)
    nc = _get_nc()
    LAST_RESULTS = run_bass_kernel_spmd(nc, in_maps, list(range(N_CORES)))
    out = np.concatenate(
        [
            LAST_RESULTS.results[i]["outT"].astype(np.float32).T
            for i in range(N_CORES)
        ],
        axis=0,
    )
    return np.ascontiguousarray(out).reshape(n, L, E)


# revision 23
# speedup vs baseline: 1.1084x; 1.1084x over previous
"""MultiHeadAttention kernel for 8x TRN2 NeuronCores.

The reference module's einsum reduces the attention tensor over BOTH the
query and key axes (attn_mass = sum_{q,k} softmax(logits)_k), and softmax
rows sum to 1, so attn_mass == Lq exactly for every (batch, head). The
whole computation therefore collapses to

    out = (Lq * (V_heads @ Wv^T + bv)).reshape(N, L, E) @ Wo^T + bo

which is a single dense GEMM after folding the (block-diagonal) per-head
V-projection into the output projection:

    out = V_flat @ W_eff + b_eff
    W_eff[h*hd+a, n] = Lq * sum_b Wv[b, a] * Wo[n, h*hd+b]      (1024 x 1024)
    b_eff[n]         = Lq * sum_{h,b} Wo[n, h*hd+b] * bv[b] + bo[n]

The device kernel is the GEMM, row-sharded across 8 cores (512 rows per
core), computed in TRANSPOSED orientation: out^T[n, m] = sum_k W[k, n]
X[m, k].  PSUM bank j holds output columns j*128..(j+1)*128 on partitions
x all 512 rows on the free dim, accumulating lhsT = W-block j against
rhs = X^T k-slabs.

v2 (this file): everything rides bf16 (inputs, weights, output — PSUM
still accumulates fp32; 2e-2 tolerance leaves ~5x margin), halving HBM
traffic, and the schedule is rebuilt around the two real bottlenecks the
fp32 trace exposed:

  * HAM clock ramp: the PE runs at ~1.2 GHz until it has been
    continuously busy ~4us, and a mid-stream DMA stall re-cools it
    (the fp32 run paid ~10us at half clock after stalling).  So: a
    bf16 junk-matmul burst starts the ramp right after the preamble
    and is sized so the first real matmul's inputs have landed by the
    time it drains — the PE never idles once started.
  * DMA supply: inputs stream over THREE queues (sync HWDGE, scalar
    HWDGE, gpsimd SWDGE), each tile ordered by its consumption
    deadline.  Banks 0 and 1 are interleaved (k-offset 2) so the
    X-slab consumption rate during the arrival phase is halved.
  * Output is bf16 too (host upcasts): banks evict through the vector
    engine (bias add fused, fp32->bf16) and drain on the sync queue,
    with the last bank split into quarters to shave the tail.
"""

import numpy as np
import ml_dtypes

import concourse.bass as bass
import concourse.bacc as bacc
import concourse.mybir as mybir
from concourse.tile import TileContext
from concourse.bass_utils import run_bass_kernel_spmd

N_CORES = 8
E = 1024            # embed dim == d_model
H, HD = 16, 64      # heads, head dim
ROWS = 4096         # N * L = 2 * 2048
RPC = ROWS // N_CORES   # rows per core = 512
P = 128             # SBUF partitions
KT = E // P         # 8 contraction slabs
JT = E // P         # 8 output-column banks

# Junk-matmul warm-up burst: keeps the PE busy (HAM ramp) from preamble
# exit until the first real operands land (~3.4us at the mid p-state).
N_JUNK_512 = 9
N_JUNK_128 = 2

# MM order = availability-greedy against the measured per-queue arrival
# times (~100 B/ns per queue, three queues, data from ~8.8/9.6/9.7us):
# banks 0..2 interleave while X and the W0-W2 half-chunks land, bank 7
# runs mid-stream (its W block rides the early gpsimd queue), bank 6
# goes last (its W block is the final scalar delivery).
MM_ORDER = [
    (0, 0), (0, 1), (0, 2), (1, 0), (0, 3), (1, 1), (1, 2), (1, 3),
    (2, 0), (2, 1), (2, 2), (2, 3), (0, 4), (0, 5), (0, 6), (0, 7),
    (1, 4), (1, 5), (1, 6), (1, 7), (2, 4), (2, 5), (2, 6), (2, 7),
] + [(j, k) for j in (3, 7, 4, 5, 6) for k in range(KT)]
# Bank completion order implied by MM_ORDER (evictions follow it).
EVICT_ORDER = [0, 1, 2, 3, 7, 4, 5, 6]

_NC_CACHE = {}
LAST_RESULTS = None  # BassKernelResults of the most recent device run


def _build():
    f32 = mybir.dt.float32
    bf16 = mybir.dt.bfloat16
    nc = bacc.Bacc(None, target_bir_lowering=False)
    xs = nc.declare_dram_parameter("xs", [E, RPC], bf16, isOutput=False)
    wc = nc.declare_dram_parameter("wc", [JT * P, E], bf16, isOutput=False)
    bw = nc.declare_dram_parameter("bw", [P, JT], f32, isOutput=False)
    outT = nc.declare_dram_parameter("outT", [E, RPC], bf16, isOutput=True)

    with TileContext(nc) as tc:
        with (
            tc.tile_pool(name="xp", bufs=1) as xp,
            tc.tile_pool(name="wp", bufs=1) as wp,
            tc.tile_pool(name="bp", bufs=1) as bp,
            tc.tile_pool(name="pp", bufs=1, space="PSUM") as pp,
            tc.tile_pool(name="op", bufs=1) as op,
        ):
            # Junk tile for the warm-up burst: memset needs no DMA and runs
            # first on gpsimd, so the PE can start right after the preamble
            # (a vector-side memset was measured to delay the burst ~1us).
            wm = bp.tile([P, RPC], bf16, name="wm", tag="wm")
            nc.gpsimd.memset(wm[:], 1.0)
            bias = bp.tile([P, JT], f32, name="bias", tag="bias")

            xts = [
                xp.tile([P, RPC], bf16, name=f"x{k}", tag=f"x{k}")
                for k in range(KT)
            ]

            # W chunk tiles; wmap[(j, k)] = (tile, col offset).
            wmap = {}

            def wchunk(j, k0, k1, engine):
                t = wp.tile([P, (k1 - k0) * P], bf16, name=f"w{j}_{k0}{k1}",
                            tag=f"w{j}_{k0}{k1}")
                engine.dma_start(
                    out=t[:], in_=wc[j * P:(j + 1) * P, k0 * P:k1 * P]
                )
                for k in range(k0, k1):
                    wmap[(j, k)] = (t, (k - k0) * P)

            # --- DMA schedule ------------------------------------------
            # Ordered per queue by consumption deadline against measured
            # constants: data starts flowing ~1.6-2.4us after issue, each
            # queue sustains only ~100 B/ns under full 8-core HBM
            # contention (the per-core HBM share is the real cap), and
            # each dma_start costs ~0.65us of issue time on its engine.
            # All W blocks ride as 128KB half-chunks so a bank's first
            # k-slabs unblock the PE a full transfer earlier.
            # gpsimd SWDGE: first halves of W0/W1/W2, x6, then W7.
            wchunk(0, 0, 4, nc.gpsimd)
            wchunk(1, 0, 4, nc.gpsimd)
            wchunk(2, 0, 4, nc.gpsimd)
            nc.gpsimd.dma_start(out=xts[6][:], in_=xs[6 * P:7 * P, :])
            wchunk(7, 0, 4, nc.gpsimd)
            wchunk(7, 4, 8, nc.gpsimd)
            # sync HWDGE: x0 first (gates the first real MM), x1, x4,
            # then the W0/W1 second halves and W3/W5.
            nc.sync.dma_start(out=xts[0][:], in_=xs[0:P, :])
            nc.sync.dma_start(out=xts[1][:], in_=xs[P:2 * P, :])
            nc.sync.dma_start(out=xts[4][:], in_=xs[4 * P:5 * P, :])
            wchunk(0, 4, 8, nc.sync)
            wchunk(1, 4, 8, nc.sync)
            wchunk(3, 0, 4, nc.sync)
            wchunk(3, 4, 8, nc.sync)
            wchunk(5, 0, 4, nc.sync)
            wchunk(5, 4, 8, nc.sync)
            # scalar HWDGE: bias (tiny, unblocks evictions), its X share,
            # the W2 second half, then W4/W6.
            nc.scalar.dma_start(out=bias[:], in_=bw[:, :])
            nc.scalar.dma_start(out=xts[2][:], in_=xs[2 * P:3 * P, :])
            nc.scalar.dma_start(out=xts[3][:], in_=xs[3 * P:4 * P, :])
            nc.scalar.dma_start(out=xts[5][:], in_=xs[5 * P:6 * P, :])
            nc.scalar.dma_start(out=xts[7][:], in_=xs[7 * P:8 * P, :])
            wchunk(2, 4, 8, nc.scalar)
            wchunk(4, 0, 4, nc.scalar)
            wchunk(4, 4, 8, nc.scalar)
            wchunk(6, 0, 4, nc.scalar)
            wchunk(6, 4, 8, nc.scalar)

            ps = [
                pp.tile([P, RPC], f32, name=f"ps{j}", tag=f"ps{j}")
                for j in range(JT)
            ]

            # Warm-up burst: nonzero bf16 junk matmuls, no DMA deps.
            for i in range(N_JUNK_512):
                nc.tensor.matmul(
                    ps[i % JT], wm[:, 0:P], wm[:, :], start=True, stop=True
                )
            for i in range(N_JUNK_128):
                nc.tensor.matmul(
                    ps[(N_JUNK_512 + i) % JT][:, 0:P],
                    wm[:, 0:P], wm[:, 0:P], start=True, stop=True,
                )

            for j, k in MM_ORDER:
                t, off = wmap[(j, k)]
                nc.tensor.matmul(
                    ps[j],
                    t[:, off:off + P],
                    xts[k][:, :],
                    start=(k == 0),
                    stop=(k == KT - 1),
                )

            # Evictions in bank-completion order: fused bias add
            # fp32->bf16 on vector, out DMAs routed to whichever queue is
            # drained when the bank completes (gpsimd's input stream ends
            # first, so it carries the early banks; outputs must not jam
            # behind remaining input FIFO traffic).  The LAST bank (6)
            # evicts in halves on vector + scalar-activation with out
            # DMAs on sync + scalar in parallel, so the post-last-matmul
            # tail is one half-bank deep.
            hh = RPC // 2
            out_eng = {0: nc.gpsimd, 1: nc.gpsimd, 2: nc.gpsimd,
                       3: nc.gpsimd, 7: nc.scalar, 4: nc.sync,
                       5: nc.scalar}
            for j in EVICT_ORDER[:-1]:
                o = op.tile([P, RPC], bf16, name=f"o{j}", tag=f"o{j}")
                nc.vector.tensor_scalar_add(o[:], ps[j], bias[:, j:j + 1])
                out_eng[j].dma_start(
                    out=outT[j * P:(j + 1) * P, :], in_=o[:]
                )
            o6 = op.tile([P, RPC], bf16, name="o6", tag="o6")
            nc.vector.tensor_scalar_add(o6[:, 0:hh], ps[6][:, 0:hh],
                                        bias[:, 6:7])
            nc.sync.dma_start(out=outT[6 * P:7 * P, 0:hh], in_=o6[:, 0:hh])
            nc.scalar.add(o6[:, hh:RPC], ps[6][:, hh:RPC], bias[:, 6:7])
            nc.scalar.dma_start(out=outT[6 * P:7 * P, hh:RPC],
                                in_=o6[:, hh:RPC])
    nc.compile()
    return nc


def _get_nc():
    if "bf16" not in _NC_CACHE:
        _NC_CACHE["bf16"] = _build()
    return _NC_CACHE["bf16"]


def _prep_in_maps(V, Wv, bv, Wo, bo, lq):
    V = np.ascontiguousarray(np.asarray(V, dtype=np.float32))
    Wv64 = np.asarray(Wv, np.float64)
    Wo64 = np.asarray(Wo, np.float64)
    bv64 = np.asarray(bv, np.float64)
    bo64 = np.asarray(bo, np.float64)

    # Fold per-head V-projection + output projection + attention mass (== Lq).
    Wo_r = Wo64.reshape(E, H, HD)                       # [n, h, b]
    W_eff = lq * np.einsum("ba,nhb->han", Wv64, Wo_r, optimize=True)
    W_eff = W_eff.reshape(E, E).astype(np.float32)      # [k, n]
    b_eff = (lq * np.einsum("nhb,b->n", Wo_r, bv64) + bo64).astype(np.float32)

    # wc[j*P + p, k*P + c] = W_eff[k*P + p, j*P + c]  (lhsT blocks, natural)
    wc = np.ascontiguousarray(
        W_eff.reshape(KT, P, JT, P).transpose(2, 1, 0, 3).reshape(JT * P, E)
    ).astype(ml_dtypes.bfloat16)
    bw_blk = np.ascontiguousarray(b_eff.reshape(JT, P).T)   # [p, j] fp32

    X = V.reshape(ROWS, E)
    in_maps = []
    for i in range(N_CORES):
        xs_i = np.ascontiguousarray(
            X[i * RPC:(i + 1) * RPC, :].T.astype(ml_dtypes.bfloat16)
        )
        in_maps.append({"xs": xs_i, "wc": wc, "bw": bw_blk})
    return in_maps


def kernel(Q, K, V, Wq, bq, Wk, bk, Wv, bv, Wo, bo, **_unused):
    global LAST_RESULTS
    n, L, e = np.asarray(V).shape
    lq = float(np.asarray(Q).shape[1])
    in_maps = _prep_in_maps(V, Wv, bv, Wo, bo, lq)
    nc = _get_nc()
    LAST_RESULTS = run_bass_kernel_spmd(nc, in_maps, list(range(N_CORES)))
    out = np.concatenate(
        [
            LAST_RESULTS.results[i]["outT"].astype(np.float32).T
            for i in range(N_CORES)
        ],
        axis=0,
    )
    return np.ascontiguousarray(out).reshape(n, L, E)


# revision 24
# speedup vs baseline: 1.1175x; 1.0082x over previous
"""MultiHeadAttention kernel for 8x TRN2 NeuronCores.

The reference module's einsum reduces the attention tensor over BOTH the
query and key axes (attn_mass = sum_{q,k} softmax(logits)_k), and softmax
rows sum to 1, so attn_mass == Lq exactly for every (batch, head). The
whole computation therefore collapses to

    out = (Lq * (V_heads @ Wv^T + bv)).reshape(N, L, E) @ Wo^T + bo

which is a single dense GEMM after folding the (block-diagonal) per-head
V-projection into the output projection:

    out = V_flat @ W_eff + b_eff
    W_eff[h*hd+a, n] = Lq * sum_b Wv[b, a] * Wo[n, h*hd+b]      (1024 x 1024)
    b_eff[n]         = Lq * sum_{h,b} Wo[n, h*hd+b] * bv[b] + bo[n]

The device kernel is the GEMM, row-sharded across 8 cores (512 rows per
core), computed in TRANSPOSED orientation: out^T[n, m] = sum_k W[k, n]
X[m, k].  PSUM bank j holds output columns j*128..(j+1)*128 on partitions
x all 512 rows on the free dim, accumulating lhsT = W-block j against
rhs = X^T k-slabs.

v2 (this file): everything rides bf16 (inputs, weights, output — PSUM
still accumulates fp32; 2e-2 tolerance leaves ~5x margin), halving HBM
traffic, and the schedule is rebuilt around the two real bottlenecks the
fp32 trace exposed:

  * HAM clock ramp: the PE runs at ~1.2 GHz until it has been
    continuously busy ~4us, and a mid-stream DMA stall re-cools it
    (the fp32 run paid ~10us at half clock after stalling).  So: a
    bf16 junk-matmul burst starts the ramp right after the preamble
    and is sized so the first real matmul's inputs have landed by the
    time it drains — the PE never idles once started.
  * DMA supply: inputs stream over THREE queues (sync HWDGE, scalar
    HWDGE, gpsimd SWDGE), each tile ordered by its consumption
    deadline.  Banks 0 and 1 are interleaved (k-offset 2) so the
    X-slab consumption rate during the arrival phase is halved.
  * Output is bf16 too (host upcasts): banks evict through the vector
    engine (bias add fused, fp32->bf16) and drain on the sync queue,
    with the last bank split into quarters to shave the tail.
"""

import numpy as np
import ml_dtypes

import concourse.bass as bass
import concourse.bacc as bacc
import concourse.mybir as mybir
from concourse.tile import TileContext
from concourse.bass_utils import run_bass_kernel_spmd

N_CORES = 8
E = 1024            # embed dim == d_model
H, HD = 16, 64      # heads, head dim
ROWS = 4096         # N * L = 2 * 2048
RPC = ROWS // N_CORES   # rows per core = 512
P = 128             # SBUF partitions
KT = E // P         # 8 contraction slabs
JT = E // P         # 8 output-column banks

# Junk-matmul warm-up burst: keeps the PE busy (HAM ramp) from preamble
# exit until the first real operands land (~3.4us at the mid p-state).
N_JUNK_512 = 7
N_JUNK_128 = 2

# MM order matched to a strict two-queue input pipeline (sync/scalar
# alternate X slabs and W half-chunks, one piece landing every ~0.8us
# per queue; 16 chip-wide queues sustain much higher per-queue rates
# than 24).  Banks 0,1 lead, bank 2 follows, then the tail banks in
# arrival order; bank 6 is last.
MM_ORDER = [
    (0, 0), (0, 1), (1, 0), (1, 1), (0, 2), (1, 2), (0, 3), (1, 3),
    (0, 4), (1, 4), (0, 5), (1, 5), (2, 0), (2, 1), (2, 2), (2, 3),
    (0, 6), (1, 6), (0, 7), (1, 7), (2, 4), (2, 5), (2, 6), (2, 7),
] + [(j, k) for j in (3, 7, 4, 5, 6) for k in range(KT)]
# Bank completion order implied by MM_ORDER (evictions follow it).
EVICT_ORDER = [0, 1, 2, 3, 7, 4, 5, 6]

_NC_CACHE = {}
LAST_RESULTS = None  # BassKernelResults of the most recent device run


def _build():
    f32 = mybir.dt.float32
    bf16 = mybir.dt.bfloat16
    nc = bacc.Bacc(None, target_bir_lowering=False)
    xs = nc.declare_dram_parameter("xs", [E, RPC], bf16, isOutput=False)
    wc = nc.declare_dram_parameter("wc", [JT * P, E], bf16, isOutput=False)
    bw = nc.declare_dram_parameter("bw", [P, JT], f32, isOutput=False)
    outT = nc.declare_dram_parameter("outT", [E, RPC], bf16, isOutput=True)

    with TileContext(nc) as tc:
        with (
            tc.tile_pool(name="xp", bufs=1) as xp,
            tc.tile_pool(name="wp", bufs=1) as wp,
            tc.tile_pool(name="bp", bufs=1) as bp,
            tc.tile_pool(name="pp", bufs=1, space="PSUM") as pp,
            tc.tile_pool(name="op", bufs=1) as op,
        ):
            # Junk tile for the warm-up burst: memset needs no DMA and runs
            # first on gpsimd, so the PE can start right after the preamble
            # (a vector-side memset was measured to delay the burst ~1us).
            wm = bp.tile([P, RPC], bf16, name="wm", tag="wm")
            nc.gpsimd.memset(wm[:], 1.0)
            bias = bp.tile([P, JT], f32, name="bias", tag="bias")

            xts = [
                xp.tile([P, RPC], bf16, name=f"x{k}", tag=f"x{k}")
                for k in range(KT)
            ]

            def xslab(k):
                return xts[k][:, :]

            # W chunk tiles; wmap[(j, k)] = (tile, col offset).
            wmap = {}

            def wchunk(j, k0, k1, engine):
                t = wp.tile([P, (k1 - k0) * P], bf16, name=f"w{j}_{k0}{k1}",
                            tag=f"w{j}_{k0}{k1}")
                engine.dma_start(
                    out=t[:], in_=wc[j * P:(j + 1) * P, k0 * P:k1 * P]
                )
                for k in range(k0, k1):
                    wmap[(j, k)] = (t, (k - k0) * P)

            # --- DMA schedule ------------------------------------------
            # Strict two-queue input pipeline: the fp32 baseline showed
            # per-queue DMA rates collapse when all 8 cores run three
            # queues (24 chip-wide) but sustain ~2x more with two.  X
            # slabs and W half-chunks alternate so each bank's operands
            # land just ahead of its matmuls; gpsimd is reserved for
            # output DMAs only.
            nc.sync.dma_start(out=xts[0][:], in_=xs[0:P, :])
            wchunk(0, 0, 4, nc.sync)
            nc.sync.dma_start(out=xts[2][:], in_=xs[2 * P:3 * P, :])
            wchunk(0, 4, 8, nc.sync)
            nc.sync.dma_start(out=xts[4][:], in_=xs[4 * P:5 * P, :])
            wchunk(2, 0, 4, nc.sync)
            nc.sync.dma_start(out=xts[6][:], in_=xs[6 * P:7 * P, :])
            wchunk(2, 4, 8, nc.sync)
            wchunk(4, 0, 4, nc.sync)
            wchunk(4, 4, 8, nc.sync)
            wchunk(6, 0, 4, nc.sync)
            wchunk(6, 4, 8, nc.sync)
            nc.scalar.dma_start(out=bias[:], in_=bw[:, :])
            nc.scalar.dma_start(out=xts[1][:], in_=xs[P:2 * P, :])
            wchunk(1, 0, 4, nc.scalar)
            nc.scalar.dma_start(out=xts[3][:], in_=xs[3 * P:4 * P, :])
            wchunk(1, 4, 8, nc.scalar)
            nc.scalar.dma_start(out=xts[5][:], in_=xs[5 * P:6 * P, :])
            wchunk(3, 0, 4, nc.scalar)
            nc.scalar.dma_start(out=xts[7][:], in_=xs[7 * P:8 * P, :])
            wchunk(3, 4, 8, nc.scalar)
            wchunk(7, 0, 4, nc.scalar)
            wchunk(7, 4, 8, nc.scalar)
            wchunk(5, 0, 4, nc.scalar)
            wchunk(5, 4, 8, nc.scalar)

            ps = [
                pp.tile([P, RPC], f32, name=f"ps{j}", tag=f"ps{j}")
                for j in range(JT)
            ]

            # Warm-up burst: nonzero bf16 junk matmuls, no DMA deps.
            for i in range(N_JUNK_512):
                nc.tensor.matmul(
                    ps[i % JT], wm[:, 0:P], wm[:, :], start=True, stop=True
                )
            for i in range(N_JUNK_128):
                nc.tensor.matmul(
                    ps[(N_JUNK_512 + i) % JT][:, 0:P],
                    wm[:, 0:P], wm[:, 0:P], start=True, stop=True,
                )

            for j, k in MM_ORDER:
                t, off = wmap[(j, k)]
                nc.tensor.matmul(
                    ps[j],
                    t[:, off:off + P],
                    xslab(k),
                    start=(k == 0),
                    stop=(k == KT - 1),
                )

            # Evictions in bank-completion order: fused bias add
            # fp32->bf16 on vector, out DMAs routed to whichever queue is
            # drained when the bank completes (each engine's outs queue
            # naturally behind its remaining input FIFO traffic).  The
            # LAST bank (6) evicts split BY PARTITION (full 1KB DRAM
            # rows, unlike column halves whose 512B strided pieces
            # transfer ~2x slower) on vector + scalar-activation, with
            # out DMAs on sync + scalar in parallel.
            hp = P // 2
            out_eng = {0: nc.gpsimd, 1: nc.gpsimd, 2: nc.gpsimd,
                       3: nc.gpsimd, 7: nc.gpsimd, 4: nc.gpsimd,
                       5: nc.gpsimd}
            for j in EVICT_ORDER[:-1]:
                o = op.tile([P, RPC], bf16, name=f"o{j}", tag=f"o{j}")
                nc.vector.tensor_scalar_add(o[:], ps[j], bias[:, j:j + 1])
                out_eng[j].dma_start(
                    out=outT[j * P:(j + 1) * P, :], in_=o[:]
                )
            o6 = op.tile([P, RPC], bf16, name="o6", tag="o6")
            nc.vector.tensor_scalar_add(o6[0:hp, :], ps[6][0:hp, :],
                                        bias[0:hp, 6:7])
            nc.sync.dma_start(out=outT[6 * P:6 * P + hp, :], in_=o6[0:hp, :])
            nc.scalar.add(o6[hp:P, :], ps[6][hp:P, :], bias[hp:P, 6:7])
            nc.scalar.dma_start(out=outT[6 * P + hp:7 * P, :], in_=o6[hp:P, :])
    nc.compile()
    return nc


def _get_nc():
    if "bf16" not in _NC_CACHE:
        _NC_CACHE["bf16"] = _build()
    return _NC_CACHE["bf16"]


def _prep_in_maps(V, Wv, bv, Wo, bo, lq):
    V = np.ascontiguousarray(np.asarray(V, dtype=np.float32))
    Wv64 = np.asarray(Wv, np.float64)
    Wo64 = np.asarray(Wo, np.float64)
    bv64 = np.asarray(bv, np.float64)
    bo64 = np.asarray(bo, np.float64)

    # Fold per-head V-projection + output projection + attention mass (== Lq).
    Wo_r = Wo64.reshape(E, H, HD)                       # [n, h, b]
    W_eff = lq * np.einsum("ba,nhb->han", Wv64, Wo_r, optimize=True)
    W_eff = W_eff.reshape(E, E).astype(np.float32)      # [k, n]
    b_eff = (lq * np.einsum("nhb,b->n", Wo_r, bv64) + bo64).astype(np.float32)

    # wc[j*P + p, k*P + c] = W_eff[k*P + p, j*P + c]  (lhsT blocks, natural)
    wc = np.ascontiguousarray(
        W_eff.reshape(KT, P, JT, P).transpose(2, 1, 0, 3).reshape(JT * P, E)
    ).astype(ml_dtypes.bfloat16)
    bw_blk = np.ascontiguousarray(b_eff.reshape(JT, P).T)   # [p, j] fp32

    X = V.reshape(ROWS, E)
    in_maps = []
    for i in range(N_CORES):
        xs_i = np.ascontiguousarray(
            X[i * RPC:(i + 1) * RPC, :].T.astype(ml_dtypes.bfloat16)
        )
        in_maps.append({"xs": xs_i, "wc": wc, "bw": bw_blk})
    return in_maps


def kernel(Q, K, V, Wq, bq, Wk, bk, Wv, bv, Wo, bo, **_unused):
    global LAST_RESULTS
    n, L, e = np.asarray(V).shape
    lq = float(np.asarray(Q).shape[1])
    in_maps = _prep_in_maps(V, Wv, bv, Wo, bo, lq)
    nc = _get_nc()
    LAST_RESULTS = run_bass_kernel_spmd(nc, in_maps, list(range(N_CORES)))
    out = np.concatenate(
        [
            LAST_RESULTS.results[i]["outT"].astype(np.float32).T
            for i in range(N_CORES)
        ],
        axis=0,
    )
    return np.ascontiguousarray(out).reshape(n, L, E)
